# revision 12
# baseline (speedup 1.0000x reference)
"""DiceLoss kernel v6: fp8 inputs, DVE/PE/ACT split, host trace decode.

HBM traffic halved by shipping outputs as fp8_e4m3 (tolerance 2e-2 vs
observed ~1e-4 error); ~22us DMA is the floor. The critical path is
lab DMA -> serial DVE mask stream -> trailing PE work -> output tail,
so the engines are balanced to finish together (~26us) and the output
drains are pipelined:

- DVE (~22.7us): all 33 per-class masks from the bf16 label tile.
  20 bf16 masks (4x mode, ~510ns) feed mixed-dtype plain matmuls;
  13 fp8 masks (2x mode, ~960ns) feed fp8 DoubleRow matmuls. Masks only
  depend on the label, so they are built ahead of the x stream (pools
  hold all 33) and never stall PE.
- PE (~23us): 32-wide chunk matmuls accumulate [32,32] PSUM blocks
  whose trace is the stat. DoubleRow sums two 32-col k-groups per
  instruction (0.5 cyc/row, both operands fp8) for the 13 fp8-mask
  intersects and 23 sum-of-squares; walrus only allows DoubleRow at PE
  tile position (0,0), so DR blocks sit at partition 0 in their own
  columns while the 20 plain-matmul blocks 3-stack at partitions
  0/32/64 in a separate column range. A warmup burst of dummy DR
  matmuls ramps the PE p-state while the first DMAs land. Per class
  the (mask-free) square is emitted before the intersect so a not-yet-
  ready mask never blocks ready work in the in-order queue.
- ACT (~21us): 10 sum-of-squares (Square+accumulate) and the staged
  PSUM->SBUF copies of finished stat-block regions; each staged copy is
  followed immediately by its output DMA so only the last small region
  remains in the tail.
- GPSIMD: nothing -- walrus rejects TensorScalarPtr/TensorTensor on
  Pool and it has no PSUM access.

The host takes block traces, bincounts labels, and computes the dice.
"""
import numpy as np
import ml_dtypes
import concourse.bacc as bacc
import concourse.mybir as mybir
import concourse.tile as tile
from concourse.bass_utils import run_bass_kernel_spmd

N_CORES = 8
B, C, X, Y, Z = 2, 33, 96, 96, 96
XS = X // N_CORES            # 12 x-slices per core
P = 128
VOX = XS * Y * Z             # 110592
F = VOX // P                 # 864
W = B * F                    # 1728 columns per class per core
H = F                        # 864 half width (= one batch slab)
CW = 32                      # chunk width
NCH = W // CW                # 54 chunks (plain matmul)
NCHH = H // CW               # 27 chunks (DoubleRow, 2 k-groups each)
NP_ = (C + 1) // 2           # 17 class pairs (class 33 is zero padding)
SMOOTH = 1e-5
N_WARMUP = 40                # dummy DR matmuls to ramp the PE p-state

A4 = frozenset({1, 4, 7, 10, 13, 16, 19, 21, 24, 27, 30, 32, 2})
SQ_ACT = (0, 3, 7, 10, 13, 16, 19, 22, 26, 29)

N_DR = len(A4) + (C - len(SQ_ACT))      # 13 + 23 = 36 DoubleRow stats
N_PL = C - len(A4)                      # 20 plain stats (3-stacked)
PL0 = N_DR                              # first plain column index
N_PLCOL = (N_PL + 2) // 3               # 7 plain columns
QW = (N_DR + N_PLCOL) * CW              # PSUM f32 dumped raw


def _assignments():
    """Stat slot assignment mirrored by build and host decode."""
    int_loc, sq_loc = {}, {}
    d_col = 0
    for c in SQ_ACT:
        sq_loc[c] = ("d", d_col)
        d_col += 1
    dr = pl = 0
    for c in range(C):
        if c not in sq_loc:
            sq_loc[c] = ("dr", dr)
            dr += 1
        if c in A4:
            int_loc[c] = ("dr", dr)
            dr += 1
        else:
            int_loc[c] = ("pl", pl)
            pl += 1
    assert dr == N_DR and pl == N_PL, (dr, pl)
    return int_loc, sq_loc, d_col


_cached = {}


def _build():
    int_loc, sq_loc, n_d = _assignments()

    nc = bacc.Bacc("TRN2", target_bir_lowering=False, debug=False,
                   num_devices=N_CORES)
    f8 = mybir.dt.float8e4
    bf = mybir.dt.bfloat16
    f32 = mybir.dt.float32
    x_in = nc.dram_tensor("x", [NP_, P, 4 * H], f8, kind="ExternalInput")
    lab_in = nc.dram_tensor("lab", [P, W], bf, kind="ExternalInput")
    stats = nc.dram_tensor("stats", [P, QW + n_d], f32, kind="ExternalOutput")

    pairs = [(2 * i, 2) for i in range(C // 2)] + [(C - 1, 1)]
    with tile.TileContext(nc) as tc:
        with (
            tc.tile_pool(name="xp", bufs=8) as xp,
            tc.tile_pool(name="labp", bufs=1) as labp,
            tc.tile_pool(name="m3p", bufs=N_PL) as m3p,
            tc.tile_pool(name="m4p", bufs=len(A4)) as m4p,
            tc.tile_pool(name="asc", bufs=2) as ascp,
            tc.tile_pool(name="stat", bufs=1) as statp,
            tc.tile_pool(name="psum", bufs=1, space="PSUM") as psp,
        ):
            psq = psp.tile([P, 4096], f32)
            # PE warmup: ramp the p-state while the first DMAs stream in.
            dum = statp.tile([P, 2, 128], f8, tag="dum")
            nc.vector.memset(dum[:, :, :], 0.0)
            for _ in range(N_WARMUP):
                nc.tensor.matmul(
                    psq[0:128, 2048:2176], dum[:, :, :], dum[:, :, :],
                    start=True, stop=True, skip_group_check=True,
                    perf_mode=mybir.MatmulPerfMode.DoubleRow)

            lab_t = labp.tile([P, 2, H], bf)
            nc.sync.dma_start(lab_t[:, :, :], lab_in[:, :])
            statd = statp.tile([P, n_d], f32, tag="statd")
            statq = statp.tile([P, QW], f32, tag="statq")

            # all 33 masks up front -- they only depend on the label
            masks = {}
            for c in range(C):
                if c in A4:
                    m = m4p.tile([P, 2, H], f8, tag="m4")
                else:
                    m = m3p.tile([P, 2, H], bf, tag="m3")
                nc.vector.tensor_scalar(
                    m[:, :, :], lab_t[:, :, :], float(c), None,
                    mybir.AluOpType.is_equal)
                masks[c] = m

            dr_ctr, pl_ctr = [0], [0]

            def emit_plain(lhs_of, rhs_of):
                k = pl_ctr[0]
                pl_ctr[0] += 1
                col, q = PL0 + k // 3, k % 3
                out_ap = psq[32 * q:32 * q + 32, 32 * col:32 * col + 32]
                for j in range(NCH):
                    h, r = j // NCHH, (j % NCHH) * CW
                    nc.tensor.matmul(
                        out_ap, lhs_of(h, r), rhs_of(h, r),
                        start=(j == 0), stop=False, skip_group_check=True)

            def emit_dr(lhs_of, rhs_of):
                k = dr_ctr[0]
                dr_ctr[0] += 1
                out_ap = psq[0:32, 32 * k:32 * k + 32]
                for j in range(NCHH):
                    r = j * CW
                    nc.tensor.matmul(
                        out_ap, lhs_of(r), rhs_of(r),
                        start=(j == 0), stop=False, skip_group_check=True,
                        perf_mode=mybir.MatmulPerfMode.DoubleRow)

            dr_copied, pl_copied = [0], [0]

            def flush_dr(hi, eng=None):
                """Copy finished DR blocks to SBUF and DMA them out (SWDGE)."""
                lo = dr_copied[0]
                if hi > lo:
                    if eng is None:
                        nc.scalar.copy(statq[0:32, 32 * lo:32 * hi],
                                       psq[0:32, 32 * lo:32 * hi])
                    else:
                        nc.vector.tensor_copy(statq[0:32, 32 * lo:32 * hi],
                                              psq[0:32, 32 * lo:32 * hi])
                    nc.sync.dma_start(stats[0:32, 32 * lo:32 * hi],
                                      statq[0:32, 32 * lo:32 * hi])
                    dr_copied[0] = hi

            def flush_pl(hi_col, rows=96, eng=None):
                lo = pl_copied[0]
                if hi_col > lo:
                    a, b = 32 * (PL0 + lo), 32 * (PL0 + hi_col)
                    if eng is None:
                        nc.scalar.copy(statq[0:rows, a:b], psq[0:rows, a:b])
                    else:
                        nc.vector.tensor_copy(statq[0:rows, a:b],
                                              psq[0:rows, a:b])
                    nc.sync.dma_start(stats[0:rows, a:b], statq[0:rows, a:b])
                    pl_copied[0] = hi_col

            for c0, n in pairs:
                pp = c0 // 2
                if n == 1:
                    xt = xp.tile([P, 1, 2, H], f8, tag="xt_last")
                    nc.sync.dma_start(xt[:, :, :, :], x_in[pp, :, 0:2 * H])
                elif pp == 0:
                    xt = xp.tile([P, 2, 2, H], f8)
                    for qi in range(4):
                        nc.sync.dma_start(
                            xt[:, qi // 2, qi % 2, :],
                            x_in[pp, :, qi * H:(qi + 1) * H])
                else:
                    xt = xp.tile([P, 2, 2, H], f8)
                    nc.sync.dma_start(xt[:, :, :, :], x_in[pp, :, :])
                for qq in range(n):
                    c = c0 + qq
                    xc = xt[:, qq]        # [P, 2, H] fp8 view
                    # square first: it never waits on a mask
                    loc = sq_loc[c]
                    if loc[0] == "d":
                        ascr = ascp.tile([P, 2, H], f8)
                        nc.scalar.activation(
                            out=ascr[:, :, :], in_=xc[:, :, :],
                            func=mybir.ActivationFunctionType.Square,
                            accum_out=statd[:, loc[1]:loc[1] + 1])
                    else:
                        emit_dr(lambda r, x=xc: x[:, :, r:r + CW],
                                lambda r, x=xc: x[:, :, r:r + CW])
                    if c == max(SQ_ACT):
                        # all ACT accumulators final -- ship them now
                        nc.sync.dma_start(stats[:, QW:QW + n_d], statd[:])
                    mask = masks[c]
                    if c in A4:
                        emit_dr(lambda r, m=mask: m[:, :, r:r + CW],
                                lambda r, x=xc: x[:, :, r:r + CW])
                    else:
                        emit_plain(lambda h, r, m=mask: m[:, h, r:r + CW],
                                   lambda h, r, x=xc: x[:, h, r:r + CW])
                    # pipelined drain of finished block regions
                    if dr_ctr[0] >= 9 and dr_ctr[0] - dr_copied[0] >= 5:
                        flush_dr(dr_ctr[0] - 1)
                    if pl_ctr[0] // 3 - pl_copied[0] >= 2:
                        flush_pl(pl_ctr[0] // 3 - 1)
            # final flushes on DVE -- it is idle once the mask stream ends
            flush_dr(N_DR, eng="dve")
            full_pl = N_PL // 3
            flush_pl(full_pl, eng="dve")
            if N_PL % 3:
                a, b = 32 * (PL0 + full_pl), 32 * (PL0 + N_PLCOL)
                rows = 32 * (N_PL % 3)
                nc.vector.tensor_copy(statq[0:rows, a:b], psq[0:rows, a:b])
                nc.sync.dma_start(stats[0:rows, a:b], statq[0:rows, a:b])
    nc.compile()
    return nc


def _get_nc():
    if "nc" not in _cached:
        _cached["nc"] = _build()
    return _cached["nc"]


def kernel(outputs, label):
    nc = _get_nc()
    outputs = np.asarray(outputs)
    lab_np = np.asarray(label)
    f8 = ml_dtypes.float8_e4m3
    bf16 = ml_dtypes.bfloat16
    in_maps = []
    for k in range(N_CORES):
        xs = outputs[:, :, k * XS:(k + 1) * XS].reshape(B, C, P, F)
        xs = np.ascontiguousarray(xs.transpose(1, 2, 0, 3))   # [C, P, B, F]
        xpad = np.zeros((2 * NP_, P, B, F), xs.dtype)
        xpad[:C] = xs
        xs = xpad.reshape(NP_, 2, P, 2 * H).transpose(0, 2, 1, 3).reshape(
            NP_, P, 4 * H)
        ls = lab_np[:, k * XS:(k + 1) * XS].reshape(B, P, F)
        ls = np.ascontiguousarray(ls.transpose(1, 0, 2)).reshape(P, W)
        in_maps.append({"x": xs.astype(f8), "lab": ls.astype(bf16)})
    res = run_bass_kernel_spmd(nc, in_maps, core_ids=list(range(N_CORES)))

    int_loc, sq_loc, n_d = _assignments()
    intersect = np.zeros(C, np.float64)
    sumsq = np.zeros(C, np.float64)
    for r in res.results:
        st = r["stats"].astype(np.float64)       # [P, QW + n_d]
        for c in range(C):
            for loc, acc in ((int_loc[c], intersect), (sq_loc[c], sumsq)):
                if loc[0] == "d":
                    acc[c] += st[:, QW + loc[1]].sum()
                elif loc[0] == "dr":
                    k = loc[1]
                    acc[c] += np.trace(st[0:32, 32 * k:32 * k + 32])
                else:
                    col, q = PL0 + loc[1] // 3, loc[1] % 3
                    acc[c] += np.trace(
                        st[32 * q:32 * q + 32, 32 * col:32 * col + 32])
    labels_sum = np.bincount(
        lab_np.reshape(-1).astype(np.int64), minlength=C).astype(np.float64)
    dice = (2.0 * intersect + SMOOTH) / (sumsq + labels_sum + SMOOTH)
    return np.float32(np.mean(1.0 - dice))


# revision 13
# speedup vs baseline: 1.1467x; 1.1467x over previous
"""DiceLoss kernel v6: fp8 inputs, DVE/PE/ACT split, host trace decode.

HBM traffic halved by shipping outputs as fp8_e4m3 (tolerance 2e-2 vs
observed ~1e-4 error); ~22us DMA is the floor. The critical path is
lab DMA -> serial DVE mask stream -> trailing PE work -> output tail,
so the engines are balanced to finish together (~26us) and the output
drains are pipelined:

- DVE (~22.7us): all 33 per-class masks from the bf16 label tile.
  20 bf16 masks (4x mode, ~510ns) feed mixed-dtype plain matmuls;
  13 fp8 masks (2x mode, ~960ns) feed fp8 DoubleRow matmuls. Masks only
  depend on the label, so they are built ahead of the x stream (pools
  hold all 33) and never stall PE.
- PE (~23us): 32-wide chunk matmuls accumulate [32,32] PSUM blocks
  whose trace is the stat. DoubleRow sums two 32-col k-groups per
  instruction (0.5 cyc/row, both operands fp8) for the 13 fp8-mask
  intersects and 23 sum-of-squares; walrus only allows DoubleRow at PE
  tile position (0,0), so DR blocks sit at partition 0 in their own
  columns while the 20 plain-matmul blocks 3-stack at partitions
  0/32/64 in a separate column range. A warmup burst of dummy DR
  matmuls ramps the PE p-state while the first DMAs land. Per class
  the (mask-free) square is emitted before the intersect so a not-yet-
  ready mask never blocks ready work in the in-order queue.
- ACT (~21us): 10 sum-of-squares (Square+accumulate) and the staged
  PSUM->SBUF copies of finished stat-block regions; each staged copy is
  followed immediately by its output DMA so only the last small region
  remains in the tail.
- GPSIMD: nothing -- walrus rejects TensorScalarPtr/TensorTensor on
  Pool and it has no PSUM access.

The host takes block traces, bincounts labels, and computes the dice.
"""
import numpy as np
import ml_dtypes
import concourse.bacc as bacc
import concourse.mybir as mybir
import concourse.tile as tile
from concourse.bass_utils import run_bass_kernel_spmd

N_CORES = 8
B, C, X, Y, Z = 2, 33, 96, 96, 96
XS = X // N_CORES            # 12 x-slices per core
P = 128
VOX = XS * Y * Z             # 110592
F = VOX // P                 # 864
W = B * F                    # 1728 columns per class per core
H = F                        # 864 half width (= one batch slab)
CW = 32                      # chunk width
NCH = W // CW                # 54 chunks (plain matmul)
NCHH = H // CW               # 27 chunks (DoubleRow, 2 k-groups each)
NP_ = (C + 1) // 2           # 17 class pairs (class 33 is zero padding)
SMOOTH = 1e-5
N_WARMUP = 40                # dummy DR matmuls to ramp the PE p-state

A4 = frozenset({1, 4, 7, 10, 13, 16, 19, 21, 24, 27, 30, 32, 2})
SQ_ACT = (0, 3, 7, 10, 13, 16, 19, 22, 26, 29)

N_DR = len(A4) + (C - len(SQ_ACT))      # 13 + 23 = 36 DoubleRow stats
N_PL = C - len(A4)                      # 20 plain stats (3-stacked)
PL0 = N_DR                              # first plain column index
N_PLCOL = (N_PL + 2) // 3               # 7 plain columns
QW = (N_DR + N_PLCOL) * CW              # PSUM f32 dumped raw


def _assignments():
    """Stat slot assignment mirrored by build and host decode."""
    int_loc, sq_loc = {}, {}
    d_col = 0
    for c in SQ_ACT:
        sq_loc[c] = ("d", d_col)
        d_col += 1
    dr = pl = 0
    for c in range(C):
        if c not in sq_loc:
            sq_loc[c] = ("dr", dr)
            dr += 1
        if c in A4:
            int_loc[c] = ("dr", dr)
            dr += 1
        else:
            int_loc[c] = ("pl", pl)
            pl += 1
    assert dr == N_DR and pl == N_PL, (dr, pl)
    return int_loc, sq_loc, d_col


_cached = {}


def _build():
    int_loc, sq_loc, n_d = _assignments()

    nc = bacc.Bacc("TRN2", target_bir_lowering=False, debug=False,
                   num_devices=N_CORES)
    f8 = mybir.dt.float8e4
    bf = mybir.dt.bfloat16
    f32 = mybir.dt.float32
    x_in = nc.dram_tensor("x", [NP_, P, 4 * H], f8, kind="ExternalInput")
    lab_in = nc.dram_tensor("lab", [P, W], bf, kind="ExternalInput")
    stats = nc.dram_tensor("stats", [P, QW + n_d], f32, kind="ExternalOutput")

    pairs = [(2 * i, 2) for i in range(C // 2)] + [(C - 1, 1)]
    with tile.TileContext(nc) as tc:
        with (
            tc.tile_pool(name="xp", bufs=8) as xp,
            tc.tile_pool(name="labp", bufs=1) as labp,
            tc.tile_pool(name="m3p", bufs=N_PL) as m3p,
            tc.tile_pool(name="m4p", bufs=len(A4)) as m4p,
            tc.tile_pool(name="asc", bufs=2) as ascp,
            tc.tile_pool(name="stat", bufs=1) as statp,
            tc.tile_pool(name="psum", bufs=1, space="PSUM") as psp,
        ):
            psq = psp.tile([P, 4096], f32)
            # PE warmup: ramp the p-state while the first DMAs stream in.
            dum = statp.tile([P, 2, 128], f8, tag="dum")
            nc.vector.memset(dum[:, :, :], 0.0)
            for _ in range(N_WARMUP):
                nc.tensor.matmul(
                    psq[0:128, 2048:2176], dum[:, :, :], dum[:, :, :],
                    start=True, stop=True, skip_group_check=True,
                    perf_mode=mybir.MatmulPerfMode.DoubleRow)

            lab_t = labp.tile([P, 2, H], bf)
            nc.sync.dma_start(lab_t[:, :, :], lab_in[:, :])
            statd = statp.tile([P, n_d], f32, tag="statd")
            statq = statp.tile([P, QW], f32, tag="statq")

            # all 33 masks up front -- they only depend on the label
            masks = {}
            for c in range(C):
                if c in A4:
                    m = m4p.tile([P, 2, H], f8, tag="m4")
                else:
                    m = m3p.tile([P, 2, H], bf, tag="m3")
                nc.vector.tensor_scalar(
                    m[:, :, :], lab_t[:, :, :], float(c), None,
                    mybir.AluOpType.is_equal)
                masks[c] = m

            dr_ctr, pl_ctr = [0], [0]

            def emit_plain(lhs_of, rhs_of):
                k = pl_ctr[0]
                pl_ctr[0] += 1
                col, q = PL0 + k // 3, k % 3
                out_ap = psq[32 * q:32 * q + 32, 32 * col:32 * col + 32]
                for j in range(NCH):
                    h, r = j // NCHH, (j % NCHH) * CW
                    nc.tensor.matmul(
                        out_ap, lhs_of(h, r), rhs_of(h, r),
                        start=(j == 0), stop=False, skip_group_check=True)

            def emit_dr(lhs_of, rhs_of):
                k = dr_ctr[0]
                dr_ctr[0] += 1
                out_ap = psq[0:32, 32 * k:32 * k + 32]
                for j in range(NCHH):
                    r = j * CW
                    nc.tensor.matmul(
                        out_ap, lhs_of(r), rhs_of(r),
                        start=(j == 0), stop=False, skip_group_check=True,
                        perf_mode=mybir.MatmulPerfMode.DoubleRow)

            dr_copied, pl_copied = [0], [0]

            def flush_dr(hi, eng=None):
                """Copy finished DR blocks to SBUF and DMA them out (SWDGE)."""
                lo = dr_copied[0]
                if hi > lo:
                    if eng is None:
                        nc.scalar.copy(statq[0:32, 32 * lo:32 * hi],
                                       psq[0:32, 32 * lo:32 * hi])
                    else:
                        nc.vector.tensor_copy(statq[0:32, 32 * lo:32 * hi],
                                              psq[0:32, 32 * lo:32 * hi])
                    nc.sync.dma_start(stats[0:32, 32 * lo:32 * hi],
                                      statq[0:32, 32 * lo:32 * hi])
                    dr_copied[0] = hi

            def flush_pl(hi_col, rows=96, eng=None):
                lo = pl_copied[0]
                if hi_col > lo:
                    a, b = 32 * (PL0 + lo), 32 * (PL0 + hi_col)
                    if eng is None:
                        nc.scalar.copy(statq[0:rows, a:b], psq[0:rows, a:b])
                    else:
                        nc.vector.tensor_copy(statq[0:rows, a:b],
                                              psq[0:rows, a:b])
                    nc.sync.dma_start(stats[0:rows, a:b], statq[0:rows, a:b])
                    pl_copied[0] = hi_col

            for c0, n in pairs:
                pp = c0 // 2
                if n == 1:
                    xt = xp.tile([P, 1, 2, H], f8, tag="xt_last")
                    nc.sync.dma_start(xt[:, :, :, :], x_in[pp, :, 0:2 * H])
                elif pp == 0:
                    xt = xp.tile([P, 2, 2, H], f8)
                    for qi in range(4):
                        nc.sync.dma_start(
                            xt[:, qi // 2, qi % 2, :],
                            x_in[pp, :, qi * H:(qi + 1) * H])
                else:
                    xt = xp.tile([P, 2, 2, H], f8)
                    nc.sync.dma_start(xt[:, :, :, :], x_in[pp, :, :])
                for qq in range(n):
                    c = c0 + qq
                    xc = xt[:, qq]        # [P, 2, H] fp8 view
                    # square first: it never waits on a mask
                    loc = sq_loc[c]
                    if loc[0] == "d":
                        ascr = ascp.tile([P, 2, H], f8)
                        nc.scalar.activation(
                            out=ascr[:, :, :], in_=xc[:, :, :],
                            func=mybir.ActivationFunctionType.Square,
                            accum_out=statd[:, loc[1]:loc[1] + 1])
                    else:
                        emit_dr(lambda r, x=xc: x[:, :, r:r + CW],
                                lambda r, x=xc: x[:, :, r:r + CW])
                    if c == max(SQ_ACT):
                        # all ACT accumulators final -- ship them now
                        nc.sync.dma_start(stats[:, QW:QW + n_d], statd[:])
                    mask = masks[c]
                    if c in A4:
                        emit_dr(lambda r, m=mask: m[:, :, r:r + CW],
                                lambda r, x=xc: x[:, :, r:r + CW])
                    else:
                        emit_plain(lambda h, r, m=mask: m[:, h, r:r + CW],
                                   lambda h, r, x=xc: x[:, h, r:r + CW])
                    # pipelined drain of finished block regions
                    if dr_ctr[0] - dr_copied[0] >= 13:
                        flush_dr(dr_ctr[0] - 1)
                    if pl_ctr[0] // 3 - pl_copied[0] >= 4:
                        flush_pl(pl_ctr[0] // 3 - 1)
            # final flushes on DVE -- it is idle once the mask stream ends
            flush_dr(N_DR, eng="dve")
            full_pl = N_PL // 3
            flush_pl(full_pl, eng="dve")
            if N_PL % 3:
                a, b = 32 * (PL0 + full_pl), 32 * (PL0 + N_PLCOL)
                rows = 32 * (N_PL % 3)
                nc.vector.tensor_copy(statq[0:rows, a:b], psq[0:rows, a:b])
                nc.sync.dma_start(stats[0:rows, a:b], statq[0:rows, a:b])
    nc.compile()
    return nc


def _get_nc():
    if "nc" not in _cached:
        _cached["nc"] = _build()
    return _cached["nc"]


def kernel(outputs, label):
    nc = _get_nc()
    outputs = np.asarray(outputs)
    lab_np = np.asarray(label)
    f8 = ml_dtypes.float8_e4m3
    bf16 = ml_dtypes.bfloat16
    in_maps = []
    for k in range(N_CORES):
        xs = outputs[:, :, k * XS:(k + 1) * XS].reshape(B, C, P, F)
        xs = np.ascontiguousarray(xs.transpose(1, 2, 0, 3))   # [C, P, B, F]
        xpad = np.zeros((2 * NP_, P, B, F), xs.dtype)
        xpad[:C] = xs
        xs = xpad.reshape(NP_, 2, P, 2 * H).transpose(0, 2, 1, 3).reshape(
            NP_, P, 4 * H)
        ls = lab_np[:, k * XS:(k + 1) * XS].reshape(B, P, F)
        ls = np.ascontiguousarray(ls.transpose(1, 0, 2)).reshape(P, W)
        in_maps.append({"x": xs.astype(f8), "lab": ls.astype(bf16)})
    res = run_bass_kernel_spmd(nc, in_maps, core_ids=list(range(N_CORES)))

    int_loc, sq_loc, n_d = _assignments()
    intersect = np.zeros(C, np.float64)
    sumsq = np.zeros(C, np.float64)
    for r in res.results:
        st = r["stats"].astype(np.float64)       # [P, QW + n_d]
        for c in range(C):
            for loc, acc in ((int_loc[c], intersect), (sq_loc[c], sumsq)):
                if loc[0] == "d":
                    acc[c] += st[:, QW + loc[1]].sum()
                elif loc[0] == "dr":
                    k = loc[1]
                    acc[c] += np.trace(st[0:32, 32 * k:32 * k + 32])
                else:
                    col, q = PL0 + loc[1] // 3, loc[1] % 3
                    acc[c] += np.trace(
                        st[32 * q:32 * q + 32, 32 * col:32 * col + 32])
    labels_sum = np.bincount(
        lab_np.reshape(-1).astype(np.int64), minlength=C).astype(np.float64)
    dice = (2.0 * intersect + SMOOTH) / (sumsq + labels_sum + SMOOTH)
    return np.float32(np.mean(1.0 - dice))
